# revision 1
# baseline (speedup 1.0000x reference)
"""AttentionBlock kernel for 8x Trainium2 NeuronCores.

Strategy: data-parallel over batch (B=8 -> 1 batch element per core).
Per core everything is computed in a channel-major ("transposed") layout so
that no on-chip transposes are needed anywhere:

  x slice in HBM is [C=512, N=1024]  (exactly xs^T)
  qk projection  -> qkT[och, tok]   (lhsT = Wp tile, rhs = x tile)
  v  projection  -> v[tok, och]     (lhsT = x tile,  rhs = Wp v-columns)
  S^T[j, i]      -> lhsT = kT jtile, rhs = qT ichunk        (d contraction)
  E = exp(scale * S^T)  on ScalarE (PSUM -> SBUF)
  O^T[d, i]      -> sum_j  lhsT = v[jtile, head cols], rhs = E[jtile]
  colsums        -> a banded-ones lhsT accumulates (head, chunk) colsums
                    into rows of two [4, 512] PSUM tiles (heads 0-1 / 2-3);
                    each batched reciprocal + K=4 selector broadcast matmul
                    runs while later heads still compute, so softmax
                    normalization never stalls the in-order PE stream.
  out^T[c, tok]  -> lhsT = Wo tile, rhs = O^T  (+ bres + x residual, one DVE op)

The v bias is folded through the output projection (sum_j P = 1), as
bres = bo + Wo^T bv, computed on device with 16 tiny matmuls pinned into
the PE stream right where the last reciprocal runs.

Matmul operands are bf16 (half the HBM traffic, fast weight load, and the
matmul stream runs at the 216 ns/512-row warm cadence); the softmax
normalization chain stays f32/f32r and the residual add reads the original
fp32 x (loaded late, off the startup critical path).  PSUM working tiles
span two banks [128, 1024] so ScalarE/VectorE run one op per tile pair.
"""

import sys

sys.path.insert(0, "/opt/trn_rl_repo")

import numpy as np

import concourse.bass as bass
import concourse.tile as tile
import concourse.mybir as mybir
from concourse.bass_utils import run_bass_kernel_spmd
from bass_rust import add_dep_helper

B, C, HW = 8, 512, 1024
NH, DK = 4, 128
OCH = NH * DK * 3  # 1536
SCALE = float(DK) ** -0.5
F32 = mybir.dt.float32
F32R = mybir.dt.float32r
BF16DT = mybir.dt.bfloat16

MM_DTYPE = "bf16"  # "bf16" | "f32r"
BF16 = MM_DTYPE == "bf16"
MMDT = BF16DT if BF16 else F32R

# ---------------------------------------------------------------------------
# Walrus in this container supports only ONE embedded sync-wait per
# instruction ("Too many sync wait commands" otherwise).  Tile emits
# multi-wait instructions, so rewrite: each instruction keeps its last wait
# and gets N-1 single-wait NoOps inserted right before it on the same engine.
# ---------------------------------------------------------------------------
_wsplit_counter = [0]


def _split_multi_waits(nc):
    for fn in nc.m.functions:
        for blk in fn.blocks:
            insts = blk.instructions
            if not insts:
                continue
            new = []
            changed = False
            for inst in insts:
                si = inst.sync_info
                waits = list(si.on_wait) if si is not None and si.on_wait else []
                if len(waits) > 1:
                    changed = True
                    for w in waits[:-1]:
                        _wsplit_counter[0] += 1
                        nop = mybir.InstNoOp(
                            name=f"WSPLIT-{_wsplit_counter[0]}",
                            ins=[],
                            outs=[],
                            engine=inst.engine,
                        )
                        nop.sync_info = mybir.SyncInfo(on_wait=[w], on_update=[])
                        nc.register_instruction(nop, overwrite=True)
                        new.append(nop)
                    inst.sync_info = mybir.SyncInfo(
                        on_wait=[waits[-1]], on_update=list(si.on_update or [])
                    )
                new.append(inst)
            if changed:
                blk.instructions = new


def build_attention_nc():
    nc = bass.Bass("TRN2")
    xb = nc.dram_tensor("xb", [C, HW], MMDT, kind="ExternalInput")
    xf = nc.dram_tensor("xf", [C, HW], F32, kind="ExternalInput")
    wp = nc.dram_tensor("wp", [C, OCH], MMDT, kind="ExternalInput")
    bp = nc.dram_tensor("bp", [OCH, 1], MMDT, kind="ExternalInput")
    wo = nc.dram_tensor("wo", [NH * DK, C], MMDT, kind="ExternalInput")
    bo = nc.dram_tensor("bo", [C, 1], F32, kind="ExternalInput")
    bpf = nc.dram_tensor("bpf", [OCH, 1], F32, kind="ExternalInput")
    # colsum row-selector band: tband[:, c] = 1 iff c == 3, so the slice
    # tband[:, 3-q : 7-q] is a [128, 4] matrix whose only ones-column is q
    tband = nc.dram_tensor("tband", [128, 7], MMDT, kind="ExternalInput")
    # broadcast row-selector band: uband[k, (3-q)*128 : (4-q)*128] == (k == q)
    uband = nc.dram_tensor("uband", [4, 7 * 128], F32R, kind="ExternalInput")
    out = nc.dram_tensor("out", [C, HW], F32, kind="ExternalOutput")

    xb, xf, wp, bp, wo, bo, bpf, tband, uband, out = (
        t.ap() for t in (xb, xf, wp, bp, wo, bo, bpf, tband, uband, out)
    )

    def r(ap):
        return ap

    EXP = mybir.ActivationFunctionType.Exp
    ADD = mybir.AluOpType.add
    MUL = mybir.AluOpType.mult
    IC = [slice(0, 512), slice(512, 1024)]

    with tile.TileContext(nc) as tc:
        with (
            tc.tile_pool(name="persist", bufs=1) as persist,
            tc.tile_pool(name="epool", bufs=24) as epool,
            tc.tile_pool(name="outp", bufs=4) as outp,
            tc.tile_pool(name="psA", bufs=3, space="PSUM") as ps_main,
            tc.tile_pool(name="psC", bufs=2, space="PSUM") as ps_cs,
        ):
            # ---- persistent SBUF tensors -------------------------------
            xb_sb = [persist.tile([128, HW], MMDT, tag=f"xb{i}", name=f"xb{i}") for i in range(4)]
            xf_sb = [persist.tile([128, HW], F32, tag=f"xf{i}", name=f"xf{i}") for i in range(4)]
            wp_sb = [persist.tile([128, OCH], MMDT, tag=f"wp{i}", name=f"wp{i}") for i in range(4)]
            wo_sb = [persist.tile([128, C], MMDT, tag=f"wo{i}", name=f"wo{i}") for i in range(4)]
            # q^T / k^T per head: index h*2 + (0=q, 1=k)
            qk_sb = [persist.tile([128, HW], MMDT, tag=f"qk{i}", name=f"qk{i}") for i in range(8)]
            # v in [tok, (h, d)] layout, 8 token tiles
            v_sb = [persist.tile([128, NH * DK], MMDT, tag=f"v{i}", name=f"v{i}") for i in range(8)]
            # attention output O^T (unnormalized, then normalized in place)
            oT_sb = [persist.tile([128, HW], MMDT, tag=f"oT{i}", name=f"oT{i}") for i in range(4)]
            bqk_sb = [persist.tile([128, 1], F32, tag=f"bqk{i}", name=f"bqk{i}") for i in range(8)]
            bv_sb = [persist.tile([128, 1], MMDT, tag=f"bv{i}", name=f"bv{i}") for i in range(4)]
            bo_sb = [persist.tile([128, 1], F32, tag=f"bo{i}", name=f"bo{i}") for i in range(4)]
            bres_sb = [persist.tile([128, 1], F32, tag=f"bres{i}", name=f"bres{i}") for i in range(4)]
            t_sb = persist.tile([128, 7], MMDT, tag="t_sb", name="t_sb")
            u_sb = persist.tile([4, 7 * 128], F32R, tag="u_sb", name="u_sb")
            csr01 = persist.tile([4, 512], F32R, tag="csr01", name="csr01")
            csr23 = persist.tile([4, 512], F32R, tag="csr23", name="csr23")

            # ---- loads -------------------------------------------------
            # bf16 x first on the sync HWDGE queue (startup critical path);
            # wp per (row-tile, head col-group) on gpsimd; fp32 x afterwards
            # on gpsimd (only needed by the residual add at the very end);
            # weights/consts on the scalar queue.
            for i in range(4):
                nc.sync.dma_start(out=xb_sb[i], in_=xb[i * 128 : (i + 1) * 128, :])
            for h in range(NH):
                for kc in range(4):
                    c0 = h * 384
                    nc.gpsimd.dma_start(
                        out=wp_sb[kc][:, c0 : c0 + 384],
                        in_=wp[kc * 128 : (kc + 1) * 128, c0 : c0 + 384],
                    )
            for i in range(4):
                nc.gpsimd.dma_start(out=xf_sb[i], in_=xf[i * 128 : (i + 1) * 128, :])
            for h in range(NH):
                for qk in range(2):
                    o0 = h * 384 + qk * 128
                    nc.scalar.dma_start(
                        out=bqk_sb[h * 2 + qk], in_=bpf[o0 : o0 + 128, 0:1]
                    )
                o0 = h * 384 + 256
                nc.scalar.dma_start(out=bv_sb[h], in_=bp[o0 : o0 + 128, 0:1])
            nc.scalar.dma_start(out=t_sb, in_=tband[:, :])
            nc.scalar.dma_start(out=u_sb, in_=uband[:, :])
            for i in range(4):
                nc.scalar.dma_start(out=bo_sb[i], in_=bo[i * 128 : (i + 1) * 128, 0:1])
                nc.scalar.dma_start(out=wo_sb[i], in_=wo[i * 128 : (i + 1) * 128, :])

            # ---- phase A: q/k projection, [och, tok] layout ------------
            for h in range(NH):
                pss = [
                    ps_main.tile([128, HW], F32, tag="psA", name="psA")
                    for _ in range(2)
                ]
                for kc in range(4):
                    for qk in range(2):
                        o0 = h * 384 + qk * 128
                        for ic in range(2):
                            nc.tensor.matmul(
                                pss[qk][:, IC[ic]],
                                r(wp_sb[kc][:, o0 : o0 + 128]),
                                r(xb_sb[kc][:, IC[ic]]),
                                start=(kc == 0),
                                stop=(kc == 3),
                            )
                for qk in range(2):
                    nc.vector.tensor_scalar_add(
                        out=qk_sb[h * 2 + qk][:],
                        in0=pss[qk][:],
                        scalar1=bqk_sb[h * 2 + qk][:],
                    )

            # ---- phase B: v projection, [tok, (h, d)] layout -----------
            wp_v = [
                wp_sb[kc][:].rearrange("p (h t) -> p h t", h=4)[:, :, 256:384]
                for kc in range(4)
            ]
            for jp in range(4):
                ps = ps_main.tile([128, HW], F32, tag="psA", name="psA")
                for jh in range(2):
                    jt = jp * 2 + jh
                    for kc in range(4):
                        nc.tensor.matmul(
                            ps[:, IC[jh]],
                            r(xb_sb[kc][:, jt * 128 : (jt + 1) * 128]),
                            r(wp_v[kc]),
                            start=(kc == 0),
                            stop=(kc == 3),
                        )
                nc.vector.tensor_copy(out=v_sb[jp * 2][:], in_=ps[:, 0:512])
                nc.vector.tensor_copy(out=v_sb[jp * 2 + 1][:], in_=ps[:, 512:1024])

            # ---- phase C: attention per head ---------------------------
            psc = [
                ps_cs.tile([4, 512], F32, tag="psC", name="psC") for _ in range(2)
            ]
            csr = [csr01, csr23]
            last_cs = [None, None]

            def attention_head(h, defer_copy=False):
                half = h // 2
                q_loc = (h % 2) * 2  # row base within this half's psc tile
                qT = qk_sb[h * 2 + 0]
                kT = qk_sb[h * 2 + 1]
                E = []
                for jt in range(8):
                    ps = ps_main.tile([128, HW], F32, tag="psA", name="psA")
                    for ic in range(2):
                        nc.tensor.matmul(
                            ps[:, IC[ic]],
                            r(kT[:, jt * 128 : (jt + 1) * 128]),
                            r(qT[:, IC[ic]]),
                        )
                    e = epool.tile([128, HW], MMDT, tag="E", name="E")
                    nc.scalar.activation(out=e[:], in_=ps[:], func=EXP, scale=SCALE)
                    E.append(e)
                pso = ps_main.tile([128, HW], F32, tag="psA", name="psA")
                for jt in range(8):
                    for ic in range(2):
                        nc.tensor.matmul(
                            pso[:, IC[ic]],
                            r(v_sb[jt][:, h * 128 : (h + 1) * 128]),
                            r(E[jt][:, IC[ic]]),
                            start=(jt == 0),
                            stop=(jt == 7),
                            skip_group_check=True,
                        )
                for jt in range(8):
                    for ic in range(2):
                        q = q_loc + ic
                        mm = nc.tensor.matmul(
                            psc[half][:],
                            r(t_sb[:, 3 - q : 7 - q]),
                            r(E[jt][:, IC[ic]]),
                            start=(h % 2 == 0 and jt == 0 and ic == 0),
                            stop=(h % 2 == 1 and jt == 7 and ic == 1),
                            skip_group_check=True,
                        )
                        last_cs[half] = mm
                def o_copy():
                    nc.vector.tensor_copy(out=oT_sb[h][:], in_=pso[:])

                if defer_copy:
                    return o_copy
                o_copy()
                return None

            def normalize_half(half):
                # reciprocal of this half's colsums (overlaps later PE work)
                with nc.allow_low_precision(
                    reason="softmax denom reciprocal rounded to f32r"
                ):
                    nc.vector.reciprocal(out=csr[half][:], in_=psc[half][:])

            def broadcast_half(half):
                for hh in range(2):
                    h = half * 2 + hh
                    bc = ps_main.tile([128, HW], F32, tag="psA", name="psA")
                    for ic in range(2):
                        q = hh * 2 + ic
                        nc.tensor.matmul(
                            bc[:, IC[ic]],
                            r(u_sb[:, (3 - q) * 128 : (4 - q) * 128]),
                            r(csr[half][:]),
                        )
                    nc.vector.tensor_tensor(
                        out=oT_sb[h][:], in0=oT_sb[h][:], in1=bc[:], op=MUL
                    )

            attention_head(0)
            copy1 = attention_head(1, defer_copy=True)
            normalize_half(0)  # recip01 enters the DVE queue first
            copy1()
            attention_head(2)
            broadcast_half(0)  # csr01 long ready; no PE stall
            copy3 = attention_head(3, defer_copy=True)
            normalize_half(1)  # recip23 on DVE while PE does bres
            copy3()

            # ---- bres = bo + Wo^T @ bv (v-bias folded through out proj);
            # pinned after the last colsum so these tiny matmuls occupy the
            # PE exactly while recip23 runs.
            for kc in range(4):
                psb = ps_cs.tile([128, 1], F32, tag="psC", name="psB")
                for km in range(4):
                    bres_l = wo_sb[km][:, kc * 128 : (kc + 1) * 128]
                    bres_r = bv_sb[km][:]
                    if not BF16:
                        bres_l = bres_l.bitcast(F32)
                        bres_r = bres_r.bitcast(F32)
                    mm = nc.tensor.matmul(
                        psb[:],
                        bres_l,
                        bres_r,
                        start=(km == 0),
                        stop=(km == 3),
                    )
                    if km == 0:
                        add_dep_helper(
                            mm.ins,
                            last_cs[1].ins,
                            reason="pin bres into the recip23 window",
                        )
                nc.vector.tensor_add(
                    out=bres_sb[kc][:], in0=psb[:], in1=bo_sb[kc][:]
                )

            # ---- phase D part 1: kc0/kc1 accumulate heads 0-1 while
            # recip23 still runs on DVE (their oT are already normalized)
            def d_matmuls(ps, kc, kms, start_km, stop_km, pin=False):
                for km in kms:
                    for ic in range(2):
                        mm = nc.tensor.matmul(
                            ps[:, IC[ic]],
                            r(wo_sb[km][:, kc * 128 : (kc + 1) * 128]),
                            r(oT_sb[km][:, IC[ic]]),
                            start=(km == start_km),
                            stop=(km == stop_km),
                            skip_group_check=True,
                        )
                        if pin and km == kms[0] and ic == 0:
                            add_dep_helper(
                                mm.ins,
                                last_cs[1].ins,
                                reason="pin D part1 into the recip23 window",
                            )

            def d_finish(ps, kc):
                for ic in range(2):
                    ot = outp.tile([128, 512], F32, tag="out", name="out")
                    # out = (psum + bres) + x_residual in one DVE op
                    nc.vector.scalar_tensor_tensor(
                        out=ot[:],
                        in0=ps[:, IC[ic]],
                        scalar=bres_sb[kc][:],
                        in1=xf_sb[kc][:, IC[ic]],
                        op0=ADD,
                        op1=ADD,
                    )
                    nc.sync.dma_start(
                        out=out[kc * 128 : (kc + 1) * 128, IC[ic]], in_=ot[:]
                    )

            psD = {}
            for kc in range(2):
                psD[kc] = ps_main.tile([128, HW], F32, tag="psA", name="psA")
                d_matmuls(psD[kc], kc, [0, 1], start_km=0, stop_km=3)

            broadcast_half(1)

            for kc in range(2):
                d_matmuls(psD[kc], kc, [2, 3], start_km=0, stop_km=3)
                d_finish(psD[kc], kc)
            for kc in range(2, 4):
                ps = ps_main.tile([128, HW], F32, tag="psA", name="psA")
                d_matmuls(ps, kc, [0, 1, 2, 3], start_km=0, stop_km=3)
                d_finish(ps, kc)

    _split_multi_waits(nc)
    return nc


_NC_CACHE = {}


def _get_nc():
    if "nc" not in _NC_CACHE:
        _NC_CACHE["nc"] = build_attention_nc()
    return _NC_CACHE["nc"]


def _band_consts():
    tb = np.zeros((128, 7), dtype=mybir.dt.np(MMDT))
    tb[:, 3] = 1.0
    ub = np.zeros((4, 7 * 128), dtype=np.float32)
    for k in range(4):
        ub[k, (3 - k) * 128 : (4 - k) * 128] = 1.0
    return tb, ub


def run_sharded(x, Wp, bp, Wo, bo, **spmd_kwargs):
    """Shard over batch, run on cores 0-7, gather.  Returns ([B,C,H,W], res)."""
    mmnp = mybir.dt.np(MMDT)
    x = np.ascontiguousarray(x, dtype=np.float32)
    xbh = x.astype(mmnp)
    Wp = np.ascontiguousarray(Wp, dtype=np.float32).astype(mmnp)
    bpf_arr = np.ascontiguousarray(bp, dtype=np.float32).reshape(OCH, 1)
    bp = bpf_arr.astype(mmnp)
    Wo = np.ascontiguousarray(Wo, dtype=np.float32).astype(mmnp)
    bo = np.ascontiguousarray(bo, dtype=np.float32).reshape(C, 1)

    nc = _get_nc()
    tb, ub = _band_consts()
    in_maps = []
    for b in range(B):
        in_maps.append(
            {
                "xb": xbh[b].reshape(C, HW),
                "xf": x[b].reshape(C, HW),
                "wp": Wp,
                "bp": bp,
                "wo": Wo,
                "bo": bo,
                "bpf": bpf_arr,
                "tband": tb,
                "uband": ub,
            }
        )
    res = run_bass_kernel_spmd(nc, in_maps, core_ids=list(range(B)), **spmd_kwargs)
    h = w = int(np.sqrt(HW))
    out = np.stack([res.results[b]["out"].reshape(C, h, w) for b in range(B)])
    return out, res


def kernel(x, Wp, bp, Wo, bo):
    out, _ = run_sharded(x, Wp, bp, Wo, bo)
    return out



# revision 10
# speedup vs baseline: 1.0630x; 1.0630x over previous
"""AttentionBlock kernel for 8x Trainium2 NeuronCores — fp8 DoubleRow edition.

Strategy: data-parallel over batch (B=8 -> 1 batch element per core).
Channel-major layout throughout (no on-chip transposes), as in the bf16
baseline, but every contraction of K>=256 runs as fp8e4 DoubleRow matmuls
(two K-tiles per instruction => 2x MAC rate on the PE):

  q/k projection  [och, tok]  : K=C=512  -> 2 DR instrs per out tile
  v  projection  [tok, och]   : K=C=512  -> 2 DR instrs per out tile
  S^T = k^T q    [j, i]       : K=d=128  -> stays bf16 (no DR gain, and
                                           keeps q/k at bf16 precision)
  E = exp(scale*S^T - 1.5)    : ScalarE, fp8 out (the -1.5 keeps E < 240,
                                           the e4m3 max; cancels in softmax)
  O^T = sum_j v E             : K=j=1024 -> 4 DR instrs per out half
  colsums (banded ones lhsT)  : K=j=1024 -> DR, accumulated per head-pair
  normalize: reciprocal_approx_fast + K=4 f32r broadcast matmul + MUL
  out proj       [c, tok]     : K=d=512  -> 2 DR instrs per out tile
  + bres + x residual (fp32) in one DVE op

Host-side prep (in run_sharded): fp8 casts, DR pair-packing of x/Wqk/Wv/Wo,
bres = bo + Wo^T bv (v bias folded through the output projection).
"""

import sys

sys.path.insert(0, "/opt/trn_rl_repo")

import numpy as np

import concourse.bass as bass
import concourse.tile as tile
import concourse.mybir as mybir
from concourse.bass_utils import run_bass_kernel_spmd

B, C, HW = 8, 512, 1024
NH, DK = 4, 128
SCALE = float(DK) ** -0.5
EXPB = -3.5  # exp bias: E = exp(scale*s - 3.5); max logit*scale is ~8.5, keeps E < 240 (fp8e4 max)
F32 = mybir.dt.float32
F32R = mybir.dt.float32r
BF16 = mybir.dt.bfloat16
FP8 = mybir.dt.float8e4
DR = mybir.MatmulPerfMode.DoubleRow

# ---------------------------------------------------------------------------
# Walrus in this container supports only ONE embedded sync-wait per
# instruction ("Too many sync wait commands" otherwise).  Tile emits
# multi-wait instructions, so rewrite: each instruction keeps its last wait
# and gets N-1 single-wait NoOps inserted right before it on the same engine.
# ---------------------------------------------------------------------------
_wsplit_counter = [0]


def _split_multi_waits(nc):
    for fn in nc.m.functions:
        for blk in fn.blocks:
            insts = blk.instructions
            if not insts:
                continue
            new = []
            changed = False
            for inst in insts:
                si = inst.sync_info
                waits = list(si.on_wait) if si is not None and si.on_wait else []
                if len(waits) > 1:
                    changed = True
                    for w in waits[:-1]:
                        _wsplit_counter[0] += 1
                        nop = mybir.InstNoOp(
                            name=f"WSPLIT-{_wsplit_counter[0]}",
                            ins=[],
                            outs=[],
                            engine=inst.engine,
                        )
                        nop.sync_info = mybir.SyncInfo(on_wait=[w], on_update=[])
                        nc.register_instruction(nop, overwrite=True)
                        new.append(nop)
                    inst.sync_info = mybir.SyncInfo(
                        on_wait=[waits[-1]], on_update=list(si.on_update or [])
                    )
                new.append(inst)
            if changed:
                blk.instructions = new


def build_attention_nc():
    nc = bass.Bass("TRN2")
    # all pair-packed fp8 tensors are host-prepared in the exact SBUF layout
    xf = nc.dram_tensor("xf", [C, HW], F32R, kind="ExternalInput")
    wqk = nc.dram_tensor("wqk", [C, 1024], F32R, kind="ExternalInput")
    wv = nc.dram_tensor("wv", [C, 512], F32R, kind="ExternalInput")
    wo8 = nc.dram_tensor("wo8", [128, 2 * 1024], FP8, kind="ExternalInput")
    bqk = nc.dram_tensor("bqk", [128, 9], F32, kind="ExternalInput")
    bres = nc.dram_tensor("bres", [128, 4], F32, kind="ExternalInput")
    tb8 = nc.dram_tensor("tb8", [128, 32], FP8, kind="ExternalInput")
    ub = nc.dram_tensor("ub", [4, 7 * 128], F32R, kind="ExternalInput")
    out = nc.dram_tensor("out", [C, HW], F32, kind="ExternalOutput")

    xf, wqk, wv, wo8, bqk, bres, tb8, ub, out = (
        t.ap() for t in (xf, wqk, wv, wo8, bqk, bres, tb8, ub, out)
    )

    EXP = mybir.ActivationFunctionType.Exp
    ADD = mybir.AluOpType.add
    MUL = mybir.AluOpType.mult
    IC = [slice(0, 512), slice(512, 1024)]

    with tile.TileContext(nc) as tc:
        with (
            tc.tile_pool(name="persist", bufs=1) as persist,
            tc.tile_pool(name="epool", bufs=10) as epool,
            tc.tile_pool(name="outp", bufs=4) as outp,
            tc.tile_pool(name="psA", bufs=3, space="PSUM") as ps_main,
            tc.tile_pool(name="psC", bufs=2, space="PSUM") as ps_cs,
        ):
            # ---- persistent SBUF tensors -------------------------------
            xf_sb = [persist.tile([128, HW], F32R, tag=f"xf{i}", name=f"xf{i}") for i in range(4)]
            wqk_sb = [persist.tile([128, 1024], F32R, tag=f"wqk{i}", name=f"wqk{i}") for i in range(4)]
            wv_sb = [persist.tile([128, 512], F32R, tag=f"wv{i}", name=f"wv{i}") for i in range(4)]
            wo_sb = [persist.tile([128, 2, 512], FP8, tag=f"wo{i}", name=f"wo{i}") for i in range(2)]
            # q^T / k^T per head in bf16: index h*2 + (0=q, 1=k)
            qk_sb = [persist.tile([128, HW], F32R, tag=f"qk{i}", name=f"qk{i}") for i in range(8)]
            # v in [tok, 2 tok-tile, (h, d)] DR layout, 4 pair tiles
            v_sb = [persist.tile([128, 2, 512], FP8, tag=f"v{i}", name=f"v{i}") for i in range(4)]
            # attention output O^T (unnormalized bf16; normalized fp8 pairs)
            oT_sb = [persist.tile([128, HW], BF16, tag=f"oT{i}", name=f"oT{i}") for i in range(4)]
            o8_sb = [persist.tile([128, 2, 1024], FP8, tag=f"o8{i}", name=f"o8{i}") for i in range(2)]
            bqk_sb = persist.tile([128, 9], F32, tag="bqk", name="bqk_sb")
            bres_sb = persist.tile([128, 4], F32, tag="bres", name="bres_sb")
            tb_sb = persist.tile([128, 2, 16], FP8, tag="tb", name="tb_sb")
            u_sb = persist.tile([4, 7 * 128], F32R, tag="u_sb", name="u_sb")
            csr = [persist.tile([4, 512], F32R, tag=f"csr{i}", name=f"csr{i}") for i in range(2)]
            rsc = [persist.tile([4, 512], F32, tag=f"rsc{i}", name=f"rsc{i}") for i in range(2)]

            # ---- loads -------------------------------------------------
            # x8 pairs on the sync HWDGE queue (startup critical path);
            # weights on vector; consts + fp32 x (residual, needed only at
            # the end) on gpsimd.
            # x (f32r, doubles as the fp32 residual) is startup-critical for
            # phase A: split across the sync and gpsimd queues.
            for i in range(2):
                nc.sync.dma_start(out=xf_sb[i], in_=xf[i * 128 : (i + 1) * 128, :])
            for i in range(2, 4):
                nc.gpsimd.dma_start(out=xf_sb[i], in_=xf[i * 128 : (i + 1) * 128, :])
            # wqk in per-head och chunks so A(h0) can start early
            for hh in range(4):
                for kc in range(4):
                    nc.scalar.dma_start(
                        out=wqk_sb[kc][:, hh * 256 : (hh + 1) * 256],
                        in_=wqk[kc * 128 : (kc + 1) * 128, hh * 256 : (hh + 1) * 256],
                    )
            for kc in range(4):
                nc.scalar.dma_start(
                    out=wv_sb[kc], in_=wv[kc * 128 : (kc + 1) * 128, :]
                )
            for p in range(2):
                nc.scalar.dma_start(
                    out=wo_sb[p],
                    in_=wo8[:, p * 1024 : (p + 1) * 1024].rearrange(
                        "p (two f) -> p two f", two=2
                    ),
                )
            nc.gpsimd.dma_start(out=bqk_sb, in_=bqk[:, :])
            nc.gpsimd.dma_start(out=bres_sb, in_=bres[:, :])
            nc.gpsimd.dma_start(
                out=tb_sb, in_=tb8[:, :].rearrange("p (two f) -> p two f", two=2)
            )
            nc.gpsimd.dma_start(out=u_sb, in_=ub[:, :])

            # ---- phase A: q/k projection, [och, tok] layout, fp8 DR ----
            for h in range(NH):
                pss = [
                    ps_main.tile([128, HW], F32, tag="psA", name="psA")
                    for _ in range(2)
                ]
                for qk in range(2):
                    o0 = h * 256 + qk * 128
                    for kc in range(4):
                        for ic in range(2):
                            nc.tensor.matmul(
                                pss[qk][:, IC[ic]],
                                wqk_sb[kc][:, o0 : o0 + 128],
                                xf_sb[kc][:, IC[ic]],
                                start=(kc == 0),
                                stop=(kc == 3),
                            )
                for qk in range(2):
                    hq = h * 2 + qk
                    nc.vector.tensor_scalar_add(
                        out=qk_sb[hq][:],
                        in0=pss[qk][:],
                        scalar1=bqk_sb[:, hq : hq + 1],
                    )

            # ---- phase B: v projection, [tok, (h, d)] layout, fp8 DR ---
            for jp in range(4):
                ps = ps_main.tile([128, HW], F32, tag="psA", name="psA")
                for jh in range(2):
                    jt = jp * 2 + jh
                    for kc in range(4):
                        nc.tensor.matmul(
                            ps[:, IC[jh]],
                            xf_sb[kc][:, jt * 128 : (jt + 1) * 128],
                            wv_sb[kc][:],
                            start=(kc == 0),
                            stop=(kc == 3),
                        )
                nc.scalar.activation(
                    out=v_sb[jp][:], in_=ps[:],
                    func=mybir.ActivationFunctionType.Copy,
                )

            # ---- phase C: attention per head ---------------------------
            psc = [
                ps_cs.tile([4, 512], F32, tag="psC", name="psC") for _ in range(2)
            ]

            def attention_head(h, defer_copy=False):
                half = h // 2
                qT = qk_sb[h * 2 + 0]
                kT = qk_sb[h * 2 + 1]
                E = []
                for jp in range(4):
                    e = epool.tile([128, 2, 1024], FP8, tag="E", name="E")
                    E.append(e)
                    for sl in range(2):
                        jt = jp * 2 + sl
                        ps = ps_main.tile([128, HW], F32, tag="psA", name="psA")
                        for ic in range(2):
                            nc.tensor.matmul(
                                ps[:, IC[ic]],
                                kT[:, jt * 128 : (jt + 1) * 128],
                                qT[:, IC[ic]],
                            )
                        nc.scalar.activation(
                            out=e[:, sl, :], in_=ps[:], func=EXP,
                            scale=SCALE, bias=bqk_sb[:, 8:9],
                        )
                pso = ps_main.tile([128, HW], F32, tag="psA", name="psA")
                for ic in range(2):
                    for jp in range(4):
                        nc.tensor.matmul(
                            pso[:, IC[ic]],
                            v_sb[jp][:, :, h * 128 : (h + 1) * 128],
                            E[jp][:, :, ic * 512 : (ic + 1) * 512],
                            start=(jp == 0),
                            stop=(jp == 3),
                            perf_mode=DR,
                            skip_group_check=True,
                        )
                for jp in range(4):
                    for ic in range(2):
                        q = (h % 2) * 2 + ic
                        nc.tensor.matmul(
                            psc[half][:],
                            tb_sb[:, :, q * 4 : q * 4 + 4],
                            E[jp][:, :, ic * 512 : (ic + 1) * 512],
                            start=(h % 2 == 0 and jp == 0 and ic == 0),
                            stop=(h % 2 == 1 and jp == 3 and ic == 1),
                            perf_mode=DR,
                            skip_group_check=True,
                        )

                def o_copy():
                    nc.vector.tensor_copy(out=oT_sb[h][:], in_=pso[:])

                if defer_copy:
                    return o_copy
                o_copy()
                return None

            def normalize_half(half):
                # fast reciprocal of this half's colsums (~51 ULP is plenty)
                with nc.allow_low_precision(
                    reason="softmax denom reciprocal, approx is fine"
                ):
                    nc.vector.reciprocal(out=csr[half][:], in_=psc[half][:])

            def broadcast_half(half):
                for hh in range(2):
                    h = half * 2 + hh
                    bc = ps_main.tile([128, HW], F32, tag="psA", name="psA")
                    for ic in range(2):
                        q = hh * 2 + ic
                        nc.tensor.matmul(
                            bc[:, IC[ic]],
                            u_sb[:, (3 - q) * 128 : (4 - q) * 128],
                            csr[half][:],
                        )
                    nc.vector.tensor_tensor(
                        out=o8_sb[half][:, hh, :], in0=oT_sb[h][:], in1=bc[:],
                        op=MUL,
                    )

            attention_head(0)
            copy1 = attention_head(1, defer_copy=True)
            normalize_half(0)  # recip01 enters the DVE queue first
            copy1()
            attention_head(2)
            broadcast_half(0)  # csr01 long ready; no PE stall
            copy3 = attention_head(3, defer_copy=True)
            normalize_half(1)  # recip23 on DVE while PE does D part 1
            copy3()

            # ---- phase D: output projection + residual, fp8 DR ---------
            def d_matmuls(ps, kc, hp, start, stop):
                for ic in range(2):
                    nc.tensor.matmul(
                        ps[:, IC[ic]],
                        wo_sb[hp][:, :, kc * 128 : (kc + 1) * 128],
                        o8_sb[hp][:, :, ic * 512 : (ic + 1) * 512],
                        start=start,
                        stop=stop,
                        perf_mode=DR,
                        skip_group_check=True,
                    )

            def d_finish(ps, kc):
                for ic in range(2):
                    ot = outp.tile([128, 512], F32, tag="out", name="out")
                    # out = (psum + bres) + x_residual in one DVE op
                    nc.vector.scalar_tensor_tensor(
                        out=ot[:],
                        in0=ps[:, IC[ic]],
                        scalar=bres_sb[:, kc : kc + 1],
                        in1=xf_sb[kc][:, IC[ic]].bitcast(F32),
                        op0=ADD,
                        op1=ADD,
                    )
                    nc.sync.dma_start(
                        out=out[kc * 128 : (kc + 1) * 128, IC[ic]], in_=ot[:]
                    )

            # part 1: heads 0-1 contribution while recip23/bc(1) run
            psD = {}
            for kc in range(2):
                psD[kc] = ps_main.tile([128, HW], F32, tag="psA", name="psA")
                d_matmuls(psD[kc], kc, 0, start=True, stop=False)

            broadcast_half(1)

            for kc in range(2):
                d_matmuls(psD[kc], kc, 1, start=False, stop=True)
                d_finish(psD[kc], kc)
            for kc in range(2, 4):
                ps = ps_main.tile([128, HW], F32, tag="psA", name="psA")
                d_matmuls(ps, kc, 0, start=True, stop=False)
                d_matmuls(ps, kc, 1, start=False, stop=True)
                d_finish(ps, kc)

    _split_multi_waits(nc)
    return nc


_NC_CACHE = {}


def _get_nc():
    if "nc" not in _NC_CACHE:
        _NC_CACHE["nc"] = build_attention_nc()
    return _NC_CACHE["nc"]


def _pair_pack(a):
    """[512, F] channel-major -> [128, 2*2*F] DR pair-packed fp8 layout:
    out[part, (pair, ktile, f)] = a[pair*256 + ktile*128 + part, f]."""
    F = a.shape[1]
    return np.ascontiguousarray(
        a.reshape(2, 2, 128, F).transpose(2, 0, 1, 3).reshape(128, 4 * F)
    )


def _prep_inputs(x, Wp, bp, Wo, bo):
    f8 = mybir.dt.np(FP8)
    x = np.ascontiguousarray(x, dtype=np.float32)
    Wp = np.asarray(Wp, dtype=np.float32)
    bp = np.asarray(bp, dtype=np.float32).reshape(-1)
    Wo = np.asarray(Wo, dtype=np.float32)
    bo = np.asarray(bo, dtype=np.float32).reshape(-1)

    qk_idx = np.concatenate(
        [np.arange(h * 384, h * 384 + 256) for h in range(NH)]
    )
    v_idx = np.concatenate(
        [np.arange(h * 384 + 256, h * 384 + 384) for h in range(NH)]
    )
    wqk_f = np.ascontiguousarray(Wp[:, qk_idx])  # [512, 1024] f32
    wv_f = np.ascontiguousarray(Wp[:, v_idx])  # [512, 512] f32
    wo8 = _pair_pack(Wo).astype(f8)  # [128, 2048]  (pairs over d-rows)
    bqk_v = np.concatenate(
        [bp[qk_idx].reshape(8, 128).T, np.full((128, 1), EXPB)], axis=1
    )  # [128, 9]: col h*2+qk = bias, col 8 = exp bias const
    bqk_v = np.ascontiguousarray(bqk_v, dtype=np.float32)
    bv = bp[v_idx]
    bres = (bo + Wo.T @ bv).reshape(4, 128).T  # [128, 4], col kc
    bres = np.ascontiguousarray(bres, dtype=np.float32)

    # colsum band: per-q 4-col window (4-byte aligned) with ones at col q,
    # duplicated for both DR k-tiles
    tb = np.zeros((128, 32), dtype=f8)
    for q in range(4):
        for kt in range(2):
            tb[:, kt * 16 + q * 4 + q] = 1.0
    ubv = np.zeros((4, 7 * 128), dtype=np.float32)
    for k in range(4):
        ubv[k, (3 - k) * 128 : (4 - k) * 128] = 1.0

    return x, wqk_f, wv_f, wo8, bqk_v, bres, tb, ubv


def run_sharded(x, Wp, bp, Wo, bo, **spmd_kwargs):
    """Shard over batch, run on cores 0-7, gather.  Returns ([B,C,H,W], res)."""
    x, wqk_f, wv_f, wo8, bqk_v, bres, tb, ubv = _prep_inputs(x, Wp, bp, Wo, bo)

    nc = _get_nc()
    in_maps = []
    for b in range(B):
        xc = x[b].reshape(C, HW)
        in_maps.append(
            {
                "xf": xc,
                "wqk": wqk_f,
                "wv": wv_f,
                "wo8": wo8,
                "bqk": bqk_v,
                "bres": bres,
                "tb8": tb,
                "ub": ubv,
            }
        )
    res = run_bass_kernel_spmd(nc, in_maps, core_ids=list(range(B)), **spmd_kwargs)
    h = w = int(np.sqrt(HW))
    out = np.stack([res.results[b]["out"].reshape(C, h, w) for b in range(B)])
    return out, res


def kernel(x, Wp, bp, Wo, bo):
    out, _ = run_sharded(x, Wp, bp, Wo, bo)
    return out


# revision 11
# speedup vs baseline: 1.0902x; 1.0256x over previous
"""AttentionBlock kernel for 8x Trainium2 NeuronCores — fp16 + fp8 DoubleRow.

Strategy: data-parallel over batch (B=8 -> 1 batch element per core).
Channel-major layout throughout (no on-chip transposes):

  q/k projection  [och, tok]  : fp16 (1 cycle/row, ~fp32 precision here)
  v  projection  [tok, och]   : fp16
  S^T = k^T q    [j, i]       : fp16 (K=128: fp8 DoubleRow gives no speedup)
  E = exp(scale*S^T - 3.5)    : ScalarE, fp8e4 out (max logit*scale ~8.5,
                                the -3.5 keeps E < 240 = fp8e4 max; the
                                softmax normalization cancels it exactly)
  O^T = sum_j v E             : fp8 DoubleRow (K=1024 -> 2x MAC rate)
  colsums (banded ones lhsT)  : fp8 DoubleRow, accumulated per head-pair
  normalize: DVE reciprocal (overlapped) + K=4 fp16 broadcast matmul + MUL
  out proj       [c, tok]     : fp8 DoubleRow (K=512)
  + bres + x residual (fp32) in one DVE op

The schedule weaves the S^T matmul stream just-in-time against the ScalarE
exp stream (the phase-C pacer), filling PE slack with the remaining
projection / attention-output / output-projection work.  S psum tiles come
from a dedicated 2-buffer pool so the weave, not psum recycling, sets the
PE lead over ScalarE.

Host-side prep: fp16/fp8 casts, DR pair-packing of Wo, bres = bo + Wo^T bv
(v bias folded through the output projection since sum_j softmax = 1).
"""

import sys

sys.path.insert(0, "/opt/trn_rl_repo")

import numpy as np

import concourse.bass as bass
import concourse.tile as tile
import concourse.mybir as mybir
from concourse.bass_utils import run_bass_kernel_spmd

B, C, HW = 8, 512, 1024
NH, DK = 4, 128
SCALE = float(DK) ** -0.5
EXPB = -3.5  # exp bias: E = exp(scale*s - 3.5); max logit*scale is ~8.5, keeps E < 240 (fp8e4 max)
F32 = mybir.dt.float32
F16 = mybir.dt.float16
BF16 = mybir.dt.bfloat16
FP8 = mybir.dt.float8e4
DR = mybir.MatmulPerfMode.DoubleRow

# ---------------------------------------------------------------------------
# Walrus in this container supports only ONE embedded sync-wait per
# instruction ("Too many sync wait commands" otherwise).  Tile emits
# multi-wait instructions, so rewrite: each instruction keeps its last wait
# and gets N-1 single-wait NoOps inserted right before it on the same engine.
# ---------------------------------------------------------------------------
_wsplit_counter = [0]


def _split_multi_waits(nc):
    for fn in nc.m.functions:
        for blk in fn.blocks:
            insts = blk.instructions
            if not insts:
                continue
            new = []
            changed = False
            for inst in insts:
                si = inst.sync_info
                waits = list(si.on_wait) if si is not None and si.on_wait else []
                if len(waits) > 1:
                    changed = True
                    for w in waits[:-1]:
                        _wsplit_counter[0] += 1
                        nop = mybir.InstNoOp(
                            name=f"WSPLIT-{_wsplit_counter[0]}",
                            ins=[],
                            outs=[],
                            engine=inst.engine,
                        )
                        nop.sync_info = mybir.SyncInfo(on_wait=[w], on_update=[])
                        nc.register_instruction(nop, overwrite=True)
                        new.append(nop)
                    inst.sync_info = mybir.SyncInfo(
                        on_wait=[waits[-1]], on_update=list(si.on_update or [])
                    )
                new.append(inst)
            if changed:
                blk.instructions = new


def build_attention_nc():
    nc = bass.Bass("TRN2")
    x16 = nc.dram_tensor("x16", [C, HW], F16, kind="ExternalInput")
    xf = nc.dram_tensor("xf", [C, HW], F32, kind="ExternalInput")
    wqk = nc.dram_tensor("wqk", [C, 1024], F16, kind="ExternalInput")
    wv = nc.dram_tensor("wv", [C, 512], F16, kind="ExternalInput")
    wo8 = nc.dram_tensor("wo8", [128, 2 * 1024], FP8, kind="ExternalInput")
    bqk = nc.dram_tensor("bqk", [128, 9], F32, kind="ExternalInput")
    bres = nc.dram_tensor("bres", [128, 4], F32, kind="ExternalInput")
    tb8 = nc.dram_tensor("tb8", [128, 32], FP8, kind="ExternalInput")
    ub = nc.dram_tensor("ub", [4, 7 * 128], F16, kind="ExternalInput")
    out = nc.dram_tensor("out", [C, HW], F32, kind="ExternalOutput")

    x16, xf, wqk, wv, wo8, bqk, bres, tb8, ub, out = (
        t.ap() for t in (x16, xf, wqk, wv, wo8, bqk, bres, tb8, ub, out)
    )

    EXP = mybir.ActivationFunctionType.Exp
    ADD = mybir.AluOpType.add
    MUL = mybir.AluOpType.mult
    IC = [slice(0, 512), slice(512, 1024)]

    with tile.TileContext(nc) as tc:
        with (
            tc.tile_pool(name="persist", bufs=1) as persist,
            tc.tile_pool(name="epool", bufs=10) as epool,
            tc.tile_pool(name="outp", bufs=4) as outp,
            tc.tile_pool(name="psS", bufs=2, space="PSUM") as ps_s,
            tc.tile_pool(name="psM", bufs=1, space="PSUM") as ps_main,
            tc.tile_pool(name="psC", bufs=2, space="PSUM") as ps_cs,
        ):
            # ---- persistent SBUF tensors -------------------------------
            x_sb = [persist.tile([128, HW], F16, tag=f"x{i}", name=f"x{i}") for i in range(4)]
            xf_sb = [persist.tile([128, HW], F32, tag=f"xf{i}", name=f"xf{i}") for i in range(4)]
            wqk_sb = [persist.tile([128, 1024], F16, tag=f"wqk{i}", name=f"wqk{i}") for i in range(4)]
            wv_sb = [persist.tile([128, 512], F16, tag=f"wv{i}", name=f"wv{i}") for i in range(4)]
            wo_sb = [persist.tile([128, 2, 512], FP8, tag=f"wo{i}", name=f"wo{i}") for i in range(2)]
            # q^T / k^T per head: index h*2 + (0=q, 1=k)
            qk_sb = [persist.tile([128, HW], F16, tag=f"qk{i}", name=f"qk{i}") for i in range(8)]
            # v in [tok, 2 tok-tile, (h, d)] DR layout, 4 pair tiles
            v_sb = [persist.tile([128, 2, 512], FP8, tag=f"v{i}", name=f"v{i}") for i in range(4)]
            # attention output O^T (unnormalized bf16; normalized fp8 pairs)
            oT_sb = [persist.tile([128, HW], BF16, tag=f"oT{i}", name=f"oT{i}") for i in range(4)]
            o8_sb = [persist.tile([128, 2, 1024], FP8, tag=f"o8{i}", name=f"o8{i}") for i in range(2)]
            bqk_sb = persist.tile([128, 9], F32, tag="bqk", name="bqk_sb")
            bres_sb = persist.tile([128, 4], F32, tag="bres", name="bres_sb")
            tb_sb = persist.tile([128, 2, 16], FP8, tag="tb", name="tb_sb")
            u_sb = persist.tile([4, 7 * 128], F16, tag="u_sb", name="u_sb")
            csr = [persist.tile([4, 512], F16, tag=f"csr{i}", name=f"csr{i}") for i in range(2)]

            # ---- loads -------------------------------------------------
            # startup-critical: x16 (sync) and wqk (scalar).  Everything
            # else on gpsimd; fp32 x last (residual, needed only at the end).
            for i in range(4):
                nc.sync.dma_start(out=x_sb[i], in_=x16[i * 128 : (i + 1) * 128, :])
            for kc in range(4):
                nc.scalar.dma_start(
                    out=wqk_sb[kc], in_=wqk[kc * 128 : (kc + 1) * 128, :]
                )
            nc.gpsimd.dma_start(out=bqk_sb, in_=bqk[:, :])
            for kc in range(4):
                nc.gpsimd.dma_start(
                    out=wv_sb[kc], in_=wv[kc * 128 : (kc + 1) * 128, :]
                )
            for p in range(2):
                nc.gpsimd.dma_start(
                    out=wo_sb[p],
                    in_=wo8[:, p * 1024 : (p + 1) * 1024].rearrange(
                        "p (two f) -> p two f", two=2
                    ),
                )
            nc.gpsimd.dma_start(out=bres_sb, in_=bres[:, :])
            nc.gpsimd.dma_start(
                out=tb_sb, in_=tb8[:, :].rearrange("p (two f) -> p two f", two=2)
            )
            nc.gpsimd.dma_start(out=u_sb, in_=ub[:, :])
            for i in range(4):
                nc.gpsimd.dma_start(out=xf_sb[i], in_=xf[i * 128 : (i + 1) * 128, :])

            psc = [
                ps_cs.tile([4, 512], F32, tag="psC", name="psC") for _ in range(2)
            ]

            # ---- phase emitters ----------------------------------------
            def emit_A(h):
                """q/k projection for head h -> qk_sb[h*2], qk_sb[h*2+1]."""
                pss = [
                    ps_main.tile([128, HW], F32, tag="psM", name="psM")
                    for _ in range(2)
                ]
                for qk in range(2):
                    o0 = h * 256 + qk * 128
                    for kc in range(4):
                        for ic in range(2):
                            nc.tensor.matmul(
                                pss[qk][:, IC[ic]],
                                wqk_sb[kc][:, o0 : o0 + 128],
                                x_sb[kc][:, IC[ic]],
                                start=(kc == 0),
                                stop=(kc == 3),
                            )
                for qk in range(2):
                    hq = h * 2 + qk
                    nc.vector.tensor_scalar_add(
                        out=qk_sb[hq][:],
                        in0=pss[qk][:],
                        scalar1=bqk_sb[:, hq : hq + 1],
                    )

            def emit_B(jp):
                """v projection for token-tile pair jp -> v_sb[jp]."""
                ps = ps_main.tile([128, HW], F32, tag="psM", name="psM")
                for jh in range(2):
                    jt = jp * 2 + jh
                    for kc in range(4):
                        nc.tensor.matmul(
                            ps[:, IC[jh]],
                            x_sb[kc][:, jt * 128 : (jt + 1) * 128],
                            wv_sb[kc][:],
                            start=(kc == 0),
                            stop=(kc == 3),
                        )
                nc.vector.tensor_copy(out=v_sb[jp][:], in_=ps[:])

            E_tiles = {h: [] for h in range(NH)}

            def emit_S(h, jt):
                """S^T tile (jt) for head h + its exp into the E pair tile."""
                qT = qk_sb[h * 2 + 0]
                kT = qk_sb[h * 2 + 1]
                jp, sl = jt // 2, jt % 2
                if sl == 0:
                    e = epool.tile([128, 2, 1024], FP8, tag="E", name="E")
                    E_tiles[h].append(e)
                e = E_tiles[h][jp]
                ps = ps_s.tile([128, HW], F32, tag="psS", name="psS")
                for ic in range(2):
                    nc.tensor.matmul(
                        ps[:, IC[ic]],
                        kT[:, jt * 128 : (jt + 1) * 128],
                        qT[:, IC[ic]],
                    )
                nc.scalar.activation(
                    out=e[:, sl, :], in_=ps[:], func=EXP,
                    scale=SCALE, bias=bqk_sb[:, 8:9],
                )

            pso = {}

            def emit_PV(h, ic):
                """attn @ v half (ic) for head h, fp8 DR over token pairs."""
                if ic == 0:
                    pso[h] = ps_main.tile([128, HW], F32, tag="psM", name="psM")
                E = E_tiles[h]
                for jp in range(4):
                    nc.tensor.matmul(
                        pso[h][:, IC[ic]],
                        v_sb[jp][:, :, h * 128 : (h + 1) * 128],
                        E[jp][:, :, ic * 512 : (ic + 1) * 512],
                        start=(jp == 0),
                        stop=(jp == 3),
                        perf_mode=DR,
                        skip_group_check=True,
                    )

            def emit_cs(h):
                """colsum accumulation for head h into psc[h//2], fp8 DR."""
                half = h // 2
                E = E_tiles[h]
                for ic in range(2):
                    for jp in range(4):
                        q = (h % 2) * 2 + ic
                        nc.tensor.matmul(
                            psc[half][:],
                            tb_sb[:, :, q * 4 : q * 4 + 4],
                            E[jp][:, :, ic * 512 : (ic + 1) * 512],
                            start=(h % 2 == 0 and ic == 0 and jp == 0),
                            stop=(h % 2 == 1 and ic == 1 and jp == 3),
                            perf_mode=DR,
                            skip_group_check=True,
                        )

            def emit_oT_copy(h):
                nc.vector.tensor_copy(out=oT_sb[h][:], in_=pso[h][:])

            def emit_recip(half):
                with nc.allow_low_precision(
                    reason="softmax denom reciprocal rounded to fp16"
                ):
                    nc.vector.reciprocal(out=csr[half][:], in_=psc[half][:])

            def emit_bc(half):
                """broadcast r over partitions (fp16 K=4 matmul) + normalize."""
                for hh in range(2):
                    h = half * 2 + hh
                    bc = ps_main.tile([128, HW], F32, tag="psM", name="psM")
                    for ic in range(2):
                        q = hh * 2 + ic
                        nc.tensor.matmul(
                            bc[:, IC[ic]],
                            u_sb[:, (3 - q) * 128 : (4 - q) * 128],
                            csr[half][:],
                        )
                    nc.vector.tensor_tensor(
                        out=o8_sb[half][:, hh, :], in0=oT_sb[h][:], in1=bc[:],
                        op=MUL,
                    )

            psD = {}

            def emit_D(kc, hp, start, stop):
                if hp == 0 and start:
                    psD[kc] = ps_s.tile([128, HW], F32, tag="psS", name="psS")
                for ic in range(2):
                    nc.tensor.matmul(
                        psD[kc][:, IC[ic]],
                        wo_sb[hp][:, :, kc * 128 : (kc + 1) * 128],
                        o8_sb[hp][:, :, ic * 512 : (ic + 1) * 512],
                        start=start,
                        stop=stop,
                        perf_mode=DR,
                        skip_group_check=True,
                    )

            def emit_D_finish(kc):
                for ic in range(2):
                    ot = outp.tile([128, 512], F32, tag="out", name="out")
                    # out = (psum + bres) + x_residual in one DVE op
                    nc.vector.scalar_tensor_tensor(
                        out=ot[:],
                        in0=psD[kc][:, IC[ic]],
                        scalar=bres_sb[:, kc : kc + 1],
                        in1=xf_sb[kc][:, IC[ic]],
                        op0=ADD,
                        op1=ADD,
                    )
                    nc.sync.dma_start(
                        out=out[kc * 128 : (kc + 1) * 128, IC[ic]], in_=ot[:]
                    )

            # ---- schedule ----------------------------------------------
            # Weave the S/exp stream (ScalarE is the phase-C pacer) against
            # projection and attention-output work so the PE never waits
            # long on exp and ScalarE starts at ~6us.
            emit_A(0)
            for jt in range(0, 3):
                emit_S(0, jt)
            emit_A(1)
            for jt in range(3, 6):
                emit_S(0, jt)
            emit_A(2)
            for jt in range(6, 8):
                emit_S(0, jt)
            emit_A(3)

            emit_S(1, 0); emit_S(1, 1)
            emit_B(0)
            emit_S(1, 2); emit_S(1, 3)
            emit_B(1)
            emit_S(1, 4); emit_S(1, 5)
            emit_B(2)
            emit_S(1, 6); emit_S(1, 7)
            emit_B(3)

            emit_S(2, 0); emit_S(2, 1)
            emit_PV(0, 0)
            emit_S(2, 2); emit_S(2, 3)
            emit_PV(0, 1)
            emit_S(2, 4); emit_S(2, 5)
            emit_cs(0)
            emit_oT_copy(0)
            emit_S(2, 6); emit_S(2, 7)

            emit_S(3, 0); emit_S(3, 1)
            emit_cs(1)
            emit_recip(0)
            emit_S(3, 2); emit_S(3, 3)
            emit_PV(1, 0)
            emit_S(3, 4); emit_S(3, 5)
            emit_PV(1, 1)
            emit_oT_copy(1)
            emit_S(3, 6); emit_S(3, 7)

            emit_cs(2)
            emit_PV(2, 0)
            emit_bc(0)
            emit_PV(2, 1)
            emit_oT_copy(2)
            emit_cs(3)
            emit_recip(1)
            emit_PV(3, 0)
            emit_PV(3, 1)
            emit_oT_copy(3)

            # D part 1 (heads 0-1) fills the recip23 window
            emit_D(0, 0, start=True, stop=False)
            emit_D(1, 0, start=True, stop=False)
            emit_bc(1)
            emit_D(0, 1, start=False, stop=True)
            emit_D_finish(0)
            emit_D(1, 1, start=False, stop=True)
            emit_D_finish(1)
            for kc in (2, 3):
                emit_D(kc, 0, start=True, stop=False)
                emit_D(kc, 1, start=False, stop=True)
                emit_D_finish(kc)

    _split_multi_waits(nc)
    return nc


_NC_CACHE = {}


def _get_nc():
    if "nc" not in _NC_CACHE:
        _NC_CACHE["nc"] = build_attention_nc()
    return _NC_CACHE["nc"]


def _pair_pack(a):
    """[512, F] channel-major -> [128, 2*2*F] DR pair-packed layout:
    out[part, (pair, ktile, f)] = a[pair*256 + ktile*128 + part, f]."""
    F = a.shape[1]
    return np.ascontiguousarray(
        a.reshape(2, 2, 128, F).transpose(2, 0, 1, 3).reshape(128, 4 * F)
    )


def _prep_inputs(x, Wp, bp, Wo, bo):
    f8 = mybir.dt.np(FP8)
    f16 = mybir.dt.np(F16)
    x = np.ascontiguousarray(x, dtype=np.float32)
    Wp = np.asarray(Wp, dtype=np.float32)
    bp = np.asarray(bp, dtype=np.float32).reshape(-1)
    Wo = np.asarray(Wo, dtype=np.float32)
    bo = np.asarray(bo, dtype=np.float32).reshape(-1)

    qk_idx = np.concatenate(
        [np.arange(h * 384, h * 384 + 256) for h in range(NH)]
    )
    v_idx = np.concatenate(
        [np.arange(h * 384 + 256, h * 384 + 384) for h in range(NH)]
    )
    wqk_f = np.ascontiguousarray(Wp[:, qk_idx]).astype(f16)  # [512, 1024]
    wv_f = np.ascontiguousarray(Wp[:, v_idx]).astype(f16)  # [512, 512]
    wo8 = _pair_pack(Wo).astype(f8)  # [128, 2048]  (pairs over d-rows)
    bqk_v = np.concatenate(
        [bp[qk_idx].reshape(8, 128).T, np.full((128, 1), EXPB)], axis=1
    )  # [128, 9]: col h*2+qk = bias, col 8 = exp bias const
    bqk_v = np.ascontiguousarray(bqk_v, dtype=np.float32)
    bv = bp[v_idx]
    bres = (bo + Wo.T @ bv).reshape(4, 128).T  # [128, 4], col kc
    bres = np.ascontiguousarray(bres, dtype=np.float32)

    # colsum band: per-q 4-col window (4-byte aligned) with ones at col q,
    # duplicated for both DR k-tiles
    tb = np.zeros((128, 32), dtype=f8)
    for q in range(4):
        for kt in range(2):
            tb[:, kt * 16 + q * 4 + q] = 1.0
    ubv = np.zeros((4, 7 * 128), dtype=f16)
    for k in range(4):
        ubv[k, (3 - k) * 128 : (4 - k) * 128] = 1.0

    return x, wqk_f, wv_f, wo8, bqk_v, bres, tb, ubv


def run_sharded(x, Wp, bp, Wo, bo, **spmd_kwargs):
    """Shard over batch, run on cores 0-7, gather.  Returns ([B,C,H,W], res)."""
    x, wqk_f, wv_f, wo8, bqk_v, bres, tb, ubv = _prep_inputs(x, Wp, bp, Wo, bo)
    f16 = mybir.dt.np(F16)

    nc = _get_nc()
    in_maps = []
    for b in range(B):
        xc = x[b].reshape(C, HW)
        in_maps.append(
            {
                "x16": xc.astype(f16),
                "xf": xc,
                "wqk": wqk_f,
                "wv": wv_f,
                "wo8": wo8,
                "bqk": bqk_v,
                "bres": bres,
                "tb8": tb,
                "ub": ubv,
            }
        )
    res = run_bass_kernel_spmd(nc, in_maps, core_ids=list(range(B)), **spmd_kwargs)
    h = w = int(np.sqrt(HW))
    out = np.stack([res.results[b]["out"].reshape(C, h, w) for b in range(B)])
    return out, res


def kernel(x, Wp, bp, Wo, bo):
    out, _ = run_sharded(x, Wp, bp, Wo, bo)
    return out


# revision 15
# speedup vs baseline: 1.2355x; 1.1334x over previous
"""AttentionBlock kernel for 8x Trainium2 NeuronCores — fp16 + fp8 DoubleRow.

Strategy: data-parallel over batch (B=8 -> 1 batch element per core).
Channel-major layout throughout (no on-chip transposes):

  q/k projection  [och, tok]  : fp16 (1 cycle/row, ~fp32 precision here)
  v  projection  [tok, och]   : fp16
  S^T = k^T q    [j, i]       : fp16 (K=128: fp8 DoubleRow gives no speedup)
  E = exp(scale*S^T - 3.5)    : ScalarE, fp8e4 out (max logit*scale ~8.5,
                                the -3.5 keeps E < 240 = fp8e4 max; the
                                softmax normalization cancels it exactly)
  O^T = sum_j v E             : fp8 DoubleRow (K=1024 -> 2x MAC rate)
  colsums (banded ones lhsT)  : fp8 DoubleRow, accumulated per head-pair
  normalize: DVE reciprocal (overlapped) + K=4 fp16 broadcast matmul + MUL
  out proj       [c, tok]     : fp8 DoubleRow (K=512)
  + bres + x residual (fp32) in one DVE op

The schedule weaves the S^T matmul stream just-in-time against the ScalarE
exp stream (the phase-C pacer), filling PE slack with the remaining
projection / attention-output / output-projection work.  S psum tiles come
from a dedicated 2-buffer pool so the weave, not psum recycling, sets the
PE lead over ScalarE.

Host-side prep: fp16/fp8 casts, DR pair-packing of Wo, bres = bo + Wo^T bv
(v bias folded through the output projection since sum_j softmax = 1).
"""

import sys

sys.path.insert(0, "/opt/trn_rl_repo")

import numpy as np

import concourse.bass as bass
import concourse.tile as tile
import concourse.mybir as mybir
from concourse.bass_utils import run_bass_kernel_spmd

B, C, HW = 8, 512, 1024
NH, DK = 4, 128
SCALE = float(DK) ** -0.5
EXPB = -3.5  # exp bias: E = exp(scale*s - 3.5); max logit*scale is ~8.5, keeps E < 240 (fp8e4 max)
F32 = mybir.dt.float32
F16 = mybir.dt.float16
BF16 = mybir.dt.bfloat16
FP8 = mybir.dt.float8e4
DR = mybir.MatmulPerfMode.DoubleRow

# ---------------------------------------------------------------------------
# Walrus in this container supports only ONE embedded sync-wait per
# instruction ("Too many sync wait commands" otherwise).  Tile emits
# multi-wait instructions, so rewrite: each instruction keeps its last wait
# and gets N-1 single-wait NoOps inserted right before it on the same engine.
# ---------------------------------------------------------------------------
_wsplit_counter = [0]


def _split_multi_waits(nc):
    for fn in nc.m.functions:
        for blk in fn.blocks:
            insts = blk.instructions
            if not insts:
                continue
            new = []
            changed = False
            for inst in insts:
                si = inst.sync_info
                waits = list(si.on_wait) if si is not None and si.on_wait else []
                if len(waits) > 1:
                    changed = True
                    for w in waits[:-1]:
                        _wsplit_counter[0] += 1
                        nop = mybir.InstNoOp(
                            name=f"WSPLIT-{_wsplit_counter[0]}",
                            ins=[],
                            outs=[],
                            engine=inst.engine,
                        )
                        nop.sync_info = mybir.SyncInfo(on_wait=[w], on_update=[])
                        nc.register_instruction(nop, overwrite=True)
                        new.append(nop)
                    inst.sync_info = mybir.SyncInfo(
                        on_wait=[waits[-1]], on_update=list(si.on_update or [])
                    )
                new.append(inst)
            if changed:
                blk.instructions = new


def build_attention_nc():
    nc = bass.Bass("TRN2")
    x16 = nc.dram_tensor("x16", [C, HW], F16, kind="ExternalInput")
    xf = nc.dram_tensor("xf", [C, HW], F32, kind="ExternalInput")
    wqk = nc.dram_tensor("wqk", [C, 1024], F16, kind="ExternalInput")
    wv = nc.dram_tensor("wv", [C, 512], F16, kind="ExternalInput")
    wo8 = nc.dram_tensor("wo8", [128, 2 * 1024], FP8, kind="ExternalInput")
    bqk = nc.dram_tensor("bqk", [128, 9], F32, kind="ExternalInput")
    bres = nc.dram_tensor("bres", [128, 4], F32, kind="ExternalInput")
    tb8 = nc.dram_tensor("tb8", [128, 32], FP8, kind="ExternalInput")
    ub = nc.dram_tensor("ub", [4, 7 * 128], F16, kind="ExternalInput")
    out = nc.dram_tensor("out", [C, HW], F32, kind="ExternalOutput")

    x16, xf, wqk, wv, wo8, bqk, bres, tb8, ub, out = (
        t.ap() for t in (x16, xf, wqk, wv, wo8, bqk, bres, tb8, ub, out)
    )

    EXP = mybir.ActivationFunctionType.Exp
    ADD = mybir.AluOpType.add
    MUL = mybir.AluOpType.mult
    IC = [slice(0, 512), slice(512, 1024)]

    with tile.TileContext(nc) as tc:
        with (
            tc.tile_pool(name="persist", bufs=1) as persist,
            tc.tile_pool(name="epool", bufs=10) as epool,
            tc.tile_pool(name="outp", bufs=4) as outp,
            tc.tile_pool(name="psS", bufs=2, space="PSUM") as ps_s,
            tc.tile_pool(name="psM", bufs=2, space="PSUM") as ps_main,
            tc.tile_pool(name="psC", bufs=2, space="PSUM") as ps_cs,
        ):
            # ---- persistent SBUF tensors -------------------------------
            x_sb = [persist.tile([128, HW], F16, tag=f"x{i}", name=f"x{i}") for i in range(4)]
            xf_sb = [persist.tile([128, HW], F32, tag=f"xf{i}", name=f"xf{i}") for i in range(4)]
            wqk_sb = [persist.tile([128, 1024], F16, tag=f"wqk{i}", name=f"wqk{i}") for i in range(4)]
            wv_sb = [persist.tile([128, 512], F16, tag=f"wv{i}", name=f"wv{i}") for i in range(4)]
            wo_sb = [persist.tile([128, 2, 512], FP8, tag=f"wo{i}", name=f"wo{i}") for i in range(2)]
            # q^T / k^T per head: index h*2 + (0=q, 1=k)
            qk_sb = [persist.tile([128, HW], F16, tag=f"qk{i}", name=f"qk{i}") for i in range(8)]
            # v in [tok, 2 tok-tile, (h, d)] DR layout, 4 pair tiles
            v_sb = [persist.tile([128, 2, 512], FP8, tag=f"v{i}", name=f"v{i}") for i in range(4)]
            # attention output O^T (unnormalized bf16; normalized fp8 pairs)
            oT_sb = [persist.tile([128, HW], BF16, tag=f"oT{i}", name=f"oT{i}") for i in range(4)]
            o8_sb = [persist.tile([128, 2, 1024], FP8, tag=f"o8{i}", name=f"o8{i}") for i in range(2)]
            bqk_sb = persist.tile([128, 9], F32, tag="bqk", name="bqk_sb")
            bres_sb = persist.tile([128, 4], F32, tag="bres", name="bres_sb")
            tb_sb = persist.tile([128, 2, 16], FP8, tag="tb", name="tb_sb")
            u_sb = persist.tile([4, 7 * 128], F16, tag="u_sb", name="u_sb")
            csr = [persist.tile([4, 512], F16, tag=f"csr{i}", name=f"csr{i}") for i in range(2)]

            # ---- loads -------------------------------------------------
            # startup-critical: x16 on sync, wqk on gpsimd (both boot fast);
            # NOTHING on scalar so the exp stream starts unobstructed.
            # fp32 x last (residual, needed only at the very end).
            for i in range(4):
                nc.sync.dma_start(out=x_sb[i], in_=x16[i * 128 : (i + 1) * 128, :])
            for kc in range(4):
                nc.gpsimd.dma_start(
                    out=wqk_sb[kc], in_=wqk[kc * 128 : (kc + 1) * 128, :]
                )
            nc.gpsimd.dma_start(out=bqk_sb, in_=bqk[:, :])
            for kc in range(4):
                nc.sync.dma_start(
                    out=wv_sb[kc], in_=wv[kc * 128 : (kc + 1) * 128, :]
                )
            for p in range(2):
                nc.gpsimd.dma_start(
                    out=wo_sb[p],
                    in_=wo8[:, p * 1024 : (p + 1) * 1024].rearrange(
                        "p (two f) -> p two f", two=2
                    ),
                )
            nc.gpsimd.dma_start(out=bres_sb, in_=bres[:, :])
            nc.gpsimd.dma_start(
                out=tb_sb, in_=tb8[:, :].rearrange("p (two f) -> p two f", two=2)
            )
            nc.gpsimd.dma_start(out=u_sb, in_=ub[:, :])
            for i in range(4):
                nc.gpsimd.dma_start(out=xf_sb[i], in_=xf[i * 128 : (i + 1) * 128, :])

            psc = [
                ps_cs.tile([4, 512], F32, tag="psC", name="psC")
                for _ in range(2)
            ]

            # trivial exp on a loaded const: forces the ACT table load (and
            # its ~1.3us cost) to happen during startup instead of right
            # before the first real exp.
            atl_scratch = persist.tile([1, 1], F32, tag="atl", name="atl")
            nc.scalar.activation(
                out=atl_scratch[:], in_=bqk_sb[0:1, 8:9], func=EXP
            )

            # ---- phase emitters ----------------------------------------
            def emit_A(h):
                """q/k projection for head h -> qk_sb[h*2], qk_sb[h*2+1].
                One [128, 512] psum group per (qk, ic) so the 3-buffer main
                pool double-buffers matmuls against the DVE drains."""
                for qk in range(2):
                    o0 = h * 256 + qk * 128
                    hq = h * 2 + qk
                    for ic in range(2):
                        ps = ps_main.tile([128, 512], F32, tag="psM", name="psM")
                        for kc in range(4):
                            nc.tensor.matmul(
                                ps[:],
                                wqk_sb[kc][:, o0 : o0 + 128],
                                x_sb[kc][:, IC[ic]],
                                start=(kc == 0),
                                stop=(kc == 3),
                            )
                        nc.vector.tensor_scalar_add(
                            out=qk_sb[hq][:, IC[ic]],
                            in0=ps[:],
                            scalar1=bqk_sb[:, hq : hq + 1],
                        )

            def emit_B(jp):
                """v projection for token-tile pair jp -> v_sb[jp]."""
                for jh in range(2):
                    jt = jp * 2 + jh
                    ps = ps_main.tile([128, 512], F32, tag="psM", name="psM")
                    for kc in range(4):
                        nc.tensor.matmul(
                            ps[:],
                            x_sb[kc][:, jt * 128 : (jt + 1) * 128],
                            wv_sb[kc][:],
                            start=(kc == 0),
                            stop=(kc == 3),
                        )
                    nc.vector.tensor_copy(out=v_sb[jp][:, jh, :], in_=ps[:])

            E_tiles = {h: [] for h in range(NH)}

            def emit_S(h, jt):
                """S^T tile (jt) for head h + its exp into the E pair tile."""
                qT = qk_sb[h * 2 + 0]
                kT = qk_sb[h * 2 + 1]
                jp, sl = jt // 2, jt % 2
                if sl == 0:
                    e = epool.tile([128, 2, 1024], FP8, tag="E", name="E")
                    E_tiles[h].append(e)
                e = E_tiles[h][jp]
                ps = ps_s.tile([128, HW], F32, tag="psS", name="psS")
                for ic in range(2):
                    nc.tensor.matmul(
                        ps[:, IC[ic]],
                        kT[:, jt * 128 : (jt + 1) * 128],
                        qT[:, IC[ic]],
                    )
                nc.scalar.activation(
                    out=e[:, sl, :], in_=ps[:], func=EXP,
                    scale=SCALE, bias=bqk_sb[:, 8:9],
                )

            pso = {}

            def emit_PV(h, ic):
                """attn @ v half (ic) for head h, fp8 DR over token pairs."""
                pso[(h, ic)] = ps_main.tile([128, 512], F32, tag="psM", name="psM")
                E = E_tiles[h]
                for jp in range(4):
                    nc.tensor.matmul(
                        pso[(h, ic)][:],
                        v_sb[jp][:, :, h * 128 : (h + 1) * 128],
                        E[jp][:, :, ic * 512 : (ic + 1) * 512],
                        start=(jp == 0),
                        stop=(jp == 3),
                        perf_mode=DR,
                        skip_group_check=True,
                    )

            def emit_cs(h):
                """colsum accumulation for head h into psc[h//2], fp8 DR."""
                half = h // 2
                E = E_tiles[h]
                for ic in range(2):
                    for jp in range(4):
                        q = (h % 2) * 2 + ic
                        nc.tensor.matmul(
                            psc[half][:],
                            tb_sb[:, :, q * 4 : q * 4 + 4],
                            E[jp][:, :, ic * 512 : (ic + 1) * 512],
                            start=(h % 2 == 0 and ic == 0 and jp == 0),
                            stop=(h % 2 == 1 and ic == 1 and jp == 3),
                            perf_mode=DR,
                            skip_group_check=True,
                        )

            def emit_oT_copy(h, eng=None):
                eng = eng or nc.vector
                for ic in range(2):
                    if eng is nc.scalar:
                        eng.activation(
                            out=oT_sb[h][:, IC[ic]], in_=pso[(h, ic)][:],
                            func=mybir.ActivationFunctionType.Copy,
                        )
                    else:
                        eng.tensor_copy(
                            out=oT_sb[h][:, IC[ic]], in_=pso[(h, ic)][:]
                        )

            def emit_recip(half):
                with nc.allow_low_precision(
                    reason="softmax denom reciprocal rounded to fp16"
                ):
                    nc.vector.reciprocal(out=csr[half][:], in_=psc[half][:])

            def emit_bc(half, pool=None):
                """broadcast r over partitions (fp16 K=4 matmul) + normalize."""
                pool = pool or ps_main
                tag = "psS" if pool is ps_s else "psM"
                for hh in range(2):
                    h = half * 2 + hh
                    for ic in range(2):
                        q = hh * 2 + ic
                        bc = pool.tile([128, 512], F32, tag=tag, name="bc")
                        nc.tensor.matmul(
                            bc[:],
                            u_sb[:, (3 - q) * 128 : (4 - q) * 128],
                            csr[half][:],
                        )
                        nc.vector.tensor_tensor(
                            out=o8_sb[half][:, hh, ic * 512 : (ic + 1) * 512],
                            in0=oT_sb[h][:, IC[ic]], in1=bc[:],
                            op=MUL,
                        )

            psD = {}

            def emit_D(kc, hp, start, stop):
                for ic in range(2):
                    if hp == 0 and start:
                        psD[(kc, ic)] = ps_main.tile(
                            [128, 512], F32, tag="psM", name="psM"
                        )
                    nc.tensor.matmul(
                        psD[(kc, ic)][:],
                        wo_sb[hp][:, :, kc * 128 : (kc + 1) * 128],
                        o8_sb[hp][:, :, ic * 512 : (ic + 1) * 512],
                        start=start,
                        stop=stop,
                        perf_mode=DR,
                        skip_group_check=True,
                    )

            def emit_D_finish(kc):
                for ic in range(2):
                    ot = outp.tile([128, 512], F32, tag="out", name="out")
                    # out = (psum + bres) + x_residual in one DVE op
                    nc.vector.scalar_tensor_tensor(
                        out=ot[:],
                        in0=psD[(kc, ic)][:],
                        scalar=bres_sb[:, kc : kc + 1],
                        in1=xf_sb[kc][:, IC[ic]],
                        op0=ADD,
                        op1=ADD,
                    )
                    nc.sync.dma_start(
                        out=out[kc * 128 : (kc + 1) * 128, IC[ic]], in_=ot[:]
                    )

            # ---- schedule ----------------------------------------------
            # Weave the S/exp stream (ScalarE is the phase-C pacer) against
            # projection and attention-output work so the PE never waits
            # long on exp and ScalarE starts at ~6us.
            emit_A(0)
            for jt in range(0, 3):
                emit_S(0, jt)
            emit_A(1)
            for jt in range(3, 6):
                emit_S(0, jt)
            emit_A(2)
            for jt in range(6, 8):
                emit_S(0, jt)
            emit_A(3)

            emit_S(1, 0); emit_S(1, 1)
            emit_B(0)
            emit_S(1, 2); emit_S(1, 3)
            emit_B(1)
            emit_S(1, 4); emit_S(1, 5)
            emit_B(2)
            emit_S(1, 6); emit_S(1, 7)
            emit_B(3)

            emit_S(2, 0); emit_S(2, 1)
            emit_PV(0, 0)
            emit_S(2, 2); emit_S(2, 3)
            emit_PV(0, 1)
            emit_S(2, 4); emit_S(2, 5)
            emit_cs(0)
            emit_oT_copy(0)
            emit_S(2, 6); emit_S(2, 7)

            emit_S(3, 0); emit_S(3, 1)
            emit_cs(1)
            emit_recip(0)
            emit_S(3, 2); emit_S(3, 3)
            emit_PV(1, 0)
            emit_S(3, 4); emit_S(3, 5)
            emit_PV(1, 1)
            emit_oT_copy(1)
            emit_S(3, 6); emit_S(3, 7)

            emit_cs(2)
            emit_PV(2, 0)
            emit_PV(2, 1)
            emit_oT_copy(2)
            emit_bc(0)
            emit_cs(3)
            emit_recip(1)
            emit_PV(3, 0)
            emit_PV(3, 1)
            emit_oT_copy(3)

            # D part 1 for kc0 + PV(3) above fill the recip23 window;
            # bc(1) draws psum from the (now idle) S pool so kc0's two
            # accumulating psD tiles can stay live in the main pool.
            emit_D(0, 0, start=True, stop=False)
            emit_bc(1, pool=ps_s)
            emit_D(0, 1, start=False, stop=True)
            emit_D_finish(0)
            for kc in (1, 2, 3):
                emit_D(kc, 0, start=True, stop=False)
                emit_D(kc, 1, start=False, stop=True)
                emit_D_finish(kc)

    _split_multi_waits(nc)
    return nc


_NC_CACHE = {}


def _get_nc():
    if "nc" not in _NC_CACHE:
        _NC_CACHE["nc"] = build_attention_nc()
    return _NC_CACHE["nc"]


def _pair_pack(a):
    """[512, F] channel-major -> [128, 2*2*F] DR pair-packed layout:
    out[part, (pair, ktile, f)] = a[pair*256 + ktile*128 + part, f]."""
    F = a.shape[1]
    return np.ascontiguousarray(
        a.reshape(2, 2, 128, F).transpose(2, 0, 1, 3).reshape(128, 4 * F)
    )


def _prep_inputs(x, Wp, bp, Wo, bo):
    f8 = mybir.dt.np(FP8)
    f16 = mybir.dt.np(F16)
    x = np.ascontiguousarray(x, dtype=np.float32)
    Wp = np.asarray(Wp, dtype=np.float32)
    bp = np.asarray(bp, dtype=np.float32).reshape(-1)
    Wo = np.asarray(Wo, dtype=np.float32)
    bo = np.asarray(bo, dtype=np.float32).reshape(-1)

    qk_idx = np.concatenate(
        [np.arange(h * 384, h * 384 + 256) for h in range(NH)]
    )
    v_idx = np.concatenate(
        [np.arange(h * 384 + 256, h * 384 + 384) for h in range(NH)]
    )
    wqk_f = np.ascontiguousarray(Wp[:, qk_idx]).astype(f16)  # [512, 1024]
    wv_f = np.ascontiguousarray(Wp[:, v_idx]).astype(f16)  # [512, 512]
    wo8 = _pair_pack(Wo).astype(f8)  # [128, 2048]  (pairs over d-rows)
    bqk_v = np.concatenate(
        [bp[qk_idx].reshape(8, 128).T, np.full((128, 1), EXPB)], axis=1
    )  # [128, 9]: col h*2+qk = bias, col 8 = exp bias const
    bqk_v = np.ascontiguousarray(bqk_v, dtype=np.float32)
    bv = bp[v_idx]
    bres = (bo + Wo.T @ bv).reshape(4, 128).T  # [128, 4], col kc
    bres = np.ascontiguousarray(bres, dtype=np.float32)

    # colsum band: per-q 4-col window (4-byte aligned) with ones at col q,
    # duplicated for both DR k-tiles
    tb = np.zeros((128, 32), dtype=f8)
    for q in range(4):
        for kt in range(2):
            tb[:, kt * 16 + q * 4 + q] = 1.0
    ubv = np.zeros((4, 7 * 128), dtype=f16)
    for k in range(4):
        ubv[k, (3 - k) * 128 : (4 - k) * 128] = 1.0

    return x, wqk_f, wv_f, wo8, bqk_v, bres, tb, ubv


def run_sharded(x, Wp, bp, Wo, bo, **spmd_kwargs):
    """Shard over batch, run on cores 0-7, gather.  Returns ([B,C,H,W], res)."""
    x, wqk_f, wv_f, wo8, bqk_v, bres, tb, ubv = _prep_inputs(x, Wp, bp, Wo, bo)
    f16 = mybir.dt.np(F16)

    nc = _get_nc()
    in_maps = []
    for b in range(B):
        xc = x[b].reshape(C, HW)
        in_maps.append(
            {
                "x16": xc.astype(f16),
                "xf": xc,
                "wqk": wqk_f,
                "wv": wv_f,
                "wo8": wo8,
                "bqk": bqk_v,
                "bres": bres,
                "tb8": tb,
                "ub": ubv,
            }
        )
    res = run_bass_kernel_spmd(nc, in_maps, core_ids=list(range(B)), **spmd_kwargs)
    h = w = int(np.sqrt(HW))
    out = np.stack([res.results[b]["out"].reshape(C, h, w) for b in range(B)])
    return out, res


def kernel(x, Wp, bp, Wo, bo):
    out, _ = run_sharded(x, Wp, bp, Wo, bo)
    return out
